# revision 15
# baseline (speedup 1.0000x reference)
"""Trainium2 Bass kernel for nn_ChannelSpatialContextAttention.

Sharding: pure data-parallel - batch B=8, one image per NeuronCore.

Per core (batch dim dropped): x [512, 16384] -> y [512, 16384].
All grouped 1x1 convs are tiny channel mixes; block-diagonalized and
algebraically fused on the host:

    xc      = relu(inorm(Wc @ x))                       compress 512->32
    att_pre = A1eff @ xc        (A1eff = A1[:, :32]@K + A1[:, 32:])
    att     = relu(inorm(att_pre))
    logits  = A2 @ att
    sm*lam  = exp(logits/tau) * (lam * S / S_c)         max-free softmax
    amv     = sm*lam*v + (1-lam)*sigmoid(logits)*v      v = V @ xc
    coord   = (alpha*ah(c,h) + beta*aw(c,w)) * xc       coord attention
    fused   = relu(inorm(Fc@coord + (M@K)@xc + M@amv))  M = Ffct @ CF
    gate    = sigmoid(gw . mean(fused)) * 0.95 + 0.05
    y       = E @ fused * gate                          expand 32->512

Layout: S-sized tensors in SBUF as [128, 4096], partition p = chunk*32+c
(4 spatial chunks of 4096).  Channel mixes -> K=128 matmuls with
kron(I4, W^T) in bf16.  Cross-chunk sums -> J = kron(ones(4,4), I32).

v2 performance structure (from NTFF trace analysis of v1):
- Input stream (32MB f32) split across two DMA queues: 16 DMAs of
  [128, 4(kc), 1024] issued 8 from SP + 8 from GpSimd(SWDGE).  The
  issuing engines carry almost nothing else, so rep i+1's input DMAs
  dispatch while rep i's tail still computes -> input wire ~always busy.
- Phase A accumulates all 4 chunk-bands of one spatial offset group
  into a single [128,1024] PSUM tile (32 matmuls) -> one full-partition
  drain per group (4 total) instead of 16 band drains.
- Everything after compress is bf16: xc/att/coord/p1/p2/fused and all
  kron lhsTs -> halves PE moving-data and SBUF footprint.  Stats stay
  f32 (bn_stats reads PSUM f32 before the bf16 drain).
- logits are consumed directly from PSUM by tanh/exp (no SBUF drain).
- gate is folded into the expand lhsT (ew_g = ew * gate), so expand
  PSUM tiles drain with plain copies (alternating ACT/DVE) and the
  outputs DMA from ACT/DVE queues (16 DMAs of [128, 4096], 2MB each).
- All tile pools live at top level; PSUM is split 4 banks (phase A +
  mid) / 4 banks (expand + tiny matmuls) so rep i+1's phase A never
  waits on rep i's expand banks.
"""

import os
import numpy as np
from ml_dtypes import bfloat16 as np_bf16

import concourse.bass as bass
import concourse.tile as tile
import concourse.mybir as mybir
from concourse.bass_utils import run_bass_kernel_spmd

try:  # persistent NEFF compile cache across calls/processes (best effort)
    import jax
    jax.config.update("jax_compilation_cache_dir", "/tmp/jax_cc_cache")
    jax.config.update("jax_persistent_cache_min_compile_time_secs", 0)
except Exception:
    pass

dt = mybir.dt
AF = mybir.ActivationFunctionType
ALU = mybir.AluOpType
AX = mybir.AxisListType

NCORES = 8
C_IN = 512
M = 32
G = 4
MG = M // G
H = 128
W = 128
S = H * W
TS = 512
NCH = 4
CHS = S // NCH
EPS = 1e-5
COT_TAU = 0.8
COT_LAM = 0.7
GATE_FLOOR = 0.05

f32 = dt.float32
f32r = dt.float32r
bf16 = dt.bfloat16
f16 = dt.float16

KR_A1EFF, KR_A2, KR_V, KR_FC, KR_M, KR_MK, KR_M15 = range(7)
PP_INCG, PP_INCB, PP_ATTG, PP_ATTB, PP_FUSG, PP_FUSB, PP_GW = range(7)
CP_PG, CP_PB, CP_AH, CP_BW = range(4)


def _block_diag(w):
    g, o, i = w.shape
    out = np.zeros((g * o, g * i), np.float32)
    for k in range(g):
        out[k * o:(k + 1) * o, k * i:(k + 1) * i] = w[k]
    return out


def _kron128(w):
    """W [32,32] (out,in) -> lhsT [128,128] = kron(I4, W.T)."""
    return np.kron(np.eye(NCH, dtype=np.float32),
                   np.ascontiguousarray(w.T, dtype=np.float32))


def _newton_rsqrt(eng, pool, y_out, v_in, P, tagp, iters=2):
    """y_out [P,1] sbuf fp32 = rsqrt(v_in); no ACT table needed."""
    yi = pool.tile([P, 1], dt.int32, tag="nri" + tagp)
    eng.tensor_single_scalar(yi[:], v_in.bitcast(dt.int32), 1,
                             ALU.logical_shift_right)
    eng.tensor_scalar(yi[:], yi[:], -1, 0x5F3759DF, ALU.mult, ALU.add)
    y = yi[:].bitcast(f32)
    half = pool.tile([P, 1], f32, tag="nrh" + tagp)
    eng.tensor_scalar_mul(half[:], v_in, 0.5)
    t = pool.tile([P, 1], f32, tag="nrt" + tagp)
    for _ in range(iters):
        eng.tensor_tensor(t[:], y, y, ALU.mult)
        eng.tensor_tensor(t[:], half[:], t[:], ALU.mult)
        eng.tensor_scalar(t[:], t[:], -1.0, 1.5, ALU.mult, ALU.add)
        eng.tensor_tensor(y, y, t[:], ALU.mult)
    eng.tensor_copy(y_out, y)


def _inorm_scale_bias(nc, pool, pj_mean, pj_e2, g_ap, b_ap, tagp):
    """From J-combined [mean, E2] ([128,1] sbuf aps): returns (scale, bias)
    [128,1] sbuf aps for relu(x*scale+bias)."""
    eng = nc.vector
    var = pool.tile([128, 1], f32, tag="inv" + tagp)
    eng.scalar_tensor_tensor(var[:], pj_mean, pj_mean, pj_e2,
                             ALU.mult, ALU.subtract)
    eng.tensor_scalar(var[:], var[:], -1.0, EPS, ALU.mult, ALU.add)
    rs = pool.tile([128, 1], f32, tag="inr" + tagp)
    _newton_rsqrt(eng, pool, rs[:], var[:], 128, tagp)
    scl = pool.tile([128, 1], f32, tag="ins" + tagp)
    eng.tensor_tensor(scl[:], rs[:], g_ap, ALU.mult)
    nscl = pool.tile([128, 1], f32, tag="inn" + tagp)
    eng.tensor_scalar_mul(nscl[:], scl[:], -1.0)
    bia = pool.tile([128, 1], f32, tag="inb" + tagp)
    eng.scalar_tensor_tensor(bia[:], pj_mean, nscl[:], b_ap,
                             ALU.mult, ALU.add)
    return scl, bia


def split_multi_waits(nc):
    """This env's walrus supports at most one sync-wait per instruction:
    hoist extra waits onto same-engine NOPs inserted just before."""
    for f in nc.m.functions:
        for bb in f.blocks:
            il = bb.instructions
            out = []
            dirty = False
            for ins in il:
                si = ins.sync_info
                waits = list(si.on_wait) if si is not None else []
                if len(waits) > 1:
                    dirty = True
                    for k, w in enumerate(waits[:-1]):
                        nop = mybir.InstNoOp(
                            name=f"wsplit_{ins.name}_{k}", ins=[], outs=[])
                        nop.engine = ins.engine
                        nop.sync_info = mybir.SyncInfo(on_wait=[w],
                                                       on_update=[])
                        out.append(nop)
                    ins.sync_info = mybir.SyncInfo(
                        on_wait=[waits[-1]], on_update=list(si.on_update))
                out.append(ins)
            if dirty:
                bb.instructions = out


def _agg_c2(nc, pool, st, tagp):
    """bn_stats buffer [128, n, 6] -> c2 [128,2] = [mean/4, (var+mean^2)/4]
    (scaled so a J-sum over the 4 chunks yields full-channel mean/E2)."""
    ag = pool.tile([128, 2], f32, tag="ag" + tagp)
    nc.vector.bn_aggr(ag[:], st[:])
    c2 = pool.tile([128, 2], f32, tag="c2" + tagp)
    nc.vector.tensor_scalar_mul(c2[:, 0:1], ag[:, 0:1], 0.25)
    e2 = pool.tile([128, 1], f32, tag="e2" + tagp)
    nc.vector.scalar_tensor_tensor(e2[:], ag[:, 0:1], ag[:, 0:1], ag[:, 1:2],
                                   ALU.mult, ALU.add)
    nc.vector.tensor_scalar_mul(c2[:, 1:2], e2[:], 0.25)
    return c2


def build_program(debug=False, reps=1):
    nc = bass.Bass("TRN2", target_bir_lowering=False, debug=False,
                   num_devices=NCORES)

    x_e = nc.dram_tensor("x", [C_IN, S], f32, kind="ExternalInput")
    cw_e = nc.dram_tensor("cw", [128, 16, 128], f32, kind="ExternalInput")
    ew_e = nc.dram_tensor("ew", [128, 512], f16, kind="ExternalInput")
    kr_e = nc.dram_tensor("kr", [128, 8, 128], f16, kind="ExternalInput")
    krm_e = nc.dram_tensor("krm", [128, 128], bf16, kind="ExternalInput")
    jm_e = nc.dram_tensor("jm", [128, 128], f32, kind="ExternalInput")
    cm_e = nc.dram_tensor("cm", [32, 3, 32], f32, kind="ExternalInput")
    pp_e = nc.dram_tensor("pp", [128, 8], f32, kind="ExternalInput")
    cp_e = nc.dram_tensor("cp", [32, 6], f32, kind="ExternalInput")
    on_e = nc.dram_tensor("on", [1, 128], f32, kind="ExternalInput")
    y_e = nc.dram_tensor("y", [C_IN, S], f32, kind="ExternalOutput")
    dbg = {}
    if debug:
        for nm in ["d_xcraw", "d_xc", "d_logits",
                   "d_p1", "d_p2", "d_coord", "d_fraw", "d_fused"]:
            dbg[nm] = nc.dram_tensor(nm, [128, CHS], f32,
                                     kind="ExternalOutput")
        dbg["d_small"] = nc.dram_tensor("d_small", [128, 16], f32,
                                        kind="ExternalOutput")

    # input DMA source view: [p, kc, ch, o4, j] with j the contiguous dim
    x_view = x_e.ap().rearrange("(k p) (c o j) -> p k c o j",
                                k=4, c=NCH, o=4)

    with tile.TileContext(nc) as tc:
      with tc.tile_pool(name="wpool", bufs=1) as wp, \
           tc.tile_pool(name="stream", bufs=5) as strm, \
           tc.tile_pool(name="scratch", bufs=1) as scr, \
           tc.tile_pool(name="texp", bufs=3) as tep, \
           tc.tile_pool(name="ybuf", bufs=2) as ybp, \
           tc.tile_pool(name="small", bufs=1) as sm, \
           tc.tile_pool(name="psA", bufs=2, space="PSUM") as psA, \
           tc.tile_pool(name="psC", bufs=2, space="PSUM") as psC:
        # ------------- weights / params (loaded once) ------
        cw = wp.tile([128, 16, 128], f32r, tag="cw")
        nc.sync.dma_start(cw[:], cw_e.ap().bitcast(f32r))
        ew = wp.tile([128, 512], f16, tag="ew")
        nc.sync.dma_start(ew[:], ew_e.ap())
        kr = wp.tile([128, 8, 128], f16, tag="kr")
        nc.sync.dma_start(kr[:], kr_e.ap())
        krm = wp.tile([128, 128], bf16, tag="krm")
        nc.sync.dma_start(krm[:], krm_e.ap())
        jm = wp.tile([128, 128], f32, tag="jm")
        nc.sync.dma_start(jm[:], jm_e.ap())
        cm = wp.tile([32, 3, 32], f32r, tag="cm")
        nc.sync.dma_start(cm[:], cm_e.ap().bitcast(f32r))
        pp = wp.tile([128, 8], f32, tag="pp")
        nc.sync.dma_start(pp[:], pp_e.ap())
        cp = wp.tile([32, 6], f32, tag="cp")
        nc.sync.dma_start(cp[:], cp_e.ap())
        ones_t = wp.tile([1, 128], f32, tag="ones_t")
        nc.sync.dma_start(ones_t[:], on_e.ap())

        for rep_i in range(reps):
            # ------------- phase A: stream + compress + stats -----------
            # 16 input DMAs of [128, 4(kc), 1024] (2MB), split across the
            # SP and GpSimd queues.  Each [128,1024] PSUM tile accumulates
            # all 4 chunk-bands x 4 kc (32 matmuls) of one offset group.
            xcraw = scr.tile([128, CHS], f16, tag="xcraw")
            stA = sm.tile([128, 8, 6], f32)
            for o4 in range(4):
                ps = psA.tile([128, 1024], f32, tag="mm")
                for ch in range(NCH):
                    xin = strm.tile([128, 4, 1024], f32r, tag="xin")
                    src = x_view[:, :, ch, o4, :].bitcast(f32r)
                    # queue balance: SP carries the first 10 input tiles
                    # (it is free at rep start), the Pool/SWDGE queue the
                    # last 6 (its ring drains rep i-1's output share first).
                    if o4 * 4 + ch < 10:
                        nc.sync.dma_start(xin[:], src)
                    else:
                        nc.gpsimd.dma_start(xin[:], src)
                    for kc in range(4):
                        for h in range(2):
                            nc.tensor.matmul(
                                ps[:, h * TS:(h + 1) * TS],
                                cw[:, kc * 4 + ch, :],
                                xin[:, kc, h * TS:(h + 1) * TS],
                                start=(ch == 0 and kc == 0),
                                stop=(ch == 3 and kc == 3))
                for ch in range(NCH):
                    for h in range(2):
                        nc.vector.bn_stats(
                            stA[ch * 32:(ch + 1) * 32, o4 * 2 + h, :],
                            ps[ch * 32:(ch + 1) * 32, h * TS:(h + 1) * TS])
                nc.scalar.copy(xcraw[:, o4 * 1024:(o4 + 1) * 1024], ps[:])

            # ------------- xc inorm + relu -------------
            c2a = _agg_c2(nc, sm, stA[:], "a")
            pja = psC.tile([128, 1024], f32, tag="mmC")
            nc.tensor.matmul(pja[:, 0:2], jm[:], c2a[:], start=True, stop=True)
            pja_s = sm.tile([128, 2], f32)
            nc.vector.tensor_copy(pja_s[:], pja[:, 0:2])
            scl1, bia1 = _inorm_scale_bias(nc, sm, pja_s[:, 0:1], pja_s[:, 1:2],
                                           pp[:, PP_INCG:PP_INCG + 1],
                                           pp[:, PP_INCB:PP_INCB + 1], "1")
            xc = scr.tile([128, CHS], f16, tag="xc")
            for q in range(4):
                sl = slice(q * 1024, (q + 1) * 1024)
                nc.scalar.activation(xc[:, sl], xcraw[:, sl], AF.Relu,
                                     bias=bia1[:], scale=scl1[:])
            xcf = xc[:]
            if debug:
                nc.sync.dma_start(dbg["d_xc"].ap(),
                                  xcf)  # dtype mismatch in debug only
                nc.sync.dma_start(dbg["d_small"].ap()[:, 0:1], scl1[:])
                nc.sync.dma_start(dbg["d_small"].ap()[:, 1:2], bia1[:])

            # ------------- att_pre -------------
            attpre = scr.tile([128, CHS], f16, tag="attpre")
            stB = sm.tile([128, 8, 6], f32)
            for q in range(4):
                sl = slice(q * 1024, (q + 1) * 1024)
                ps = psA.tile([128, 1024], f32, tag="mm")
                for h in range(2):
                    nc.tensor.matmul(ps[:, h * TS:(h + 1) * TS],
                                     kr[:, KR_A1EFF, :],
                                     xc[:, q * 1024 + h * TS:
                                        q * 1024 + (h + 1) * TS],
                                     start=True, stop=True)
                    nc.vector.bn_stats(stB[:, q * 2 + h, :],
                                       ps[:, h * TS:(h + 1) * TS])
                if q % 2 == 0:
                    nc.scalar.copy(attpre[:, sl], ps[:])
                else:
                    nc.vector.tensor_copy(attpre[:, sl], ps[:])
            c2b = _agg_c2(nc, sm, stB[:], "b")
            pjb = psC.tile([128, 1024], f32, tag="mmC")
            nc.tensor.matmul(pjb[:, 0:2], jm[:], c2b[:], start=True, stop=True)
            pjb_s = sm.tile([128, 2], f32)
            nc.vector.tensor_copy(pjb_s[:], pjb[:, 0:2])
            scl2, bia2 = _inorm_scale_bias(nc, sm, pjb_s[:, 0:1], pjb_s[:, 1:2],
                                           pp[:, PP_ATTG:PP_ATTG + 1],
                                           pp[:, PP_ATTB:PP_ATTB + 1], "2")
            att = scr.tile([128, CHS], f16, tag="att")
            for q in range(4):
                sl = slice(q * 1024, (q + 1) * 1024)
                nc.scalar.activation(att[:, sl], attpre[:, sl], AF.Relu,
                                     bias=bia2[:], scale=scl2[:])

            # ---- logits (PSUM-resident) + v + tanh/exp + p1/p2 ----
            # SBUF slot reuse: p1 takes xcraw's slot (dead after xc relu),
            # p2 takes attpre's (dead after att relu).
            p1 = scr.tile([128, CHS], bf16, tag="xcraw")
            p2 = scr.tile([128, CHS], f16, tag="attpre")
            esum4 = sm.tile([128, 4], f32)
            for q in range(4):
                sl = slice(q * 1024, (q + 1) * 1024)
                pslog = psA.tile([128, 1024], f32, tag="mm")
                psv = psA.tile([128, 1024], f32, tag="mm")
                for h in range(2):
                    hsl = slice(q * 1024 + h * TS, q * 1024 + (h + 1) * TS)
                    osl = slice(h * TS, (h + 1) * TS)
                    nc.tensor.matmul(pslog[:, osl], kr[:, KR_A2, :],
                                     att[:, hsl], start=True, stop=True)
                    nc.tensor.matmul(psv[:, osl], kr[:, KR_V, :],
                                     xc[:, hsl], start=True, stop=True)
                th_c = tep.tile([128, 1024], f16, tag="th")
                ex_c = tep.tile([128, 1024], bf16, tag="ex")
                nc.scalar.activation(th_c[:], pslog[:], AF.Tanh,
                                     bias=0.0, scale=0.5)
                nc.scalar.activation(ex_c[:], pslog[:], AF.Exp,
                                     bias=0.0, scale=1.0 / COT_TAU,
                                     accum_out=esum4[:, q:q + 1])
                nc.vector.tensor_tensor(p1[:, sl], ex_c[:], psv[:],
                                        ALU.mult)
                nc.vector.scalar_tensor_tensor(p2[:, sl], th_c[:],
                                               1.0, psv[:],
                                               ALU.add, ALU.mult)

            esumS = sm.tile([128, 1], f32)
            nc.vector.tensor_reduce(esumS[:], esum4[:], axis=AX.X, op=ALU.add)
            pS = psC.tile([128, 1024], f32, tag="mmC")
            nc.tensor.matmul(pS[:, 0:1], jm[:], esumS[:], start=True,
                             stop=True)
            recS = sm.tile([128, 1], f32)
            nc.vector.reciprocal(recS[:], pS[:, 0:1])
            smscl = sm.tile([128, 1], f32)
            nc.vector.tensor_scalar_mul(smscl[:], recS[:], COT_LAM * S)
            # krs = M_kron * smscl (per-contraction-row scale)
            krs = wp.tile([128, 128], bf16, tag="krs")
            nc.vector.tensor_scalar_mul(krs[:], krm[:], smscl[:])

            # ------------- coord branch --------
            zhp = sm.tile([128, 32], f32)
            zwp4 = sm.tile([128, 4, W], f32)
            for q in range(4):
                sl = slice(q * 1024, (q + 1) * 1024)
                nc.vector.tensor_reduce(
                    zhp[:, q * 8:(q + 1) * 8],
                    xcf[:, sl].rearrange("p (a b) -> p a b", b=W),
                    axis=AX.X, op=ALU.add)
                nc.vector.tensor_reduce(
                    zwp4[:, q, :],
                    xcf[:, sl].rearrange("p (a b) -> p b a", b=W),
                    axis=AX.X, op=ALU.add)
            zwp = sm.tile([128, W], f32)
            nc.vector.tensor_reduce(
                zwp[:], zwp4[:].rearrange("p q w -> p w q"),
                axis=AX.X, op=ALU.add)
            zhps = sm.tile([128, 32], f32)
            nc.vector.tensor_scalar_mul(zhps[:], zhp[:], 1.0 / W)
            zh_c = sm.tile([32, H], f32r)
            for ch in range(NCH):
                nc.sync.dma_start(zh_c[:, ch * 32:(ch + 1) * 32],
                                  zhps[ch * 32:(ch + 1) * 32, :].bitcast(f32r))
            pzw = psC.tile([128, 1024], f32, tag="mmC")
            nc.tensor.matmul(pzw[:, 0:W], jm[:], zwp[:], start=True, stop=True)
            zw_c = sm.tile([32, W], f32r)
            nc.vector.tensor_scalar_mul(zw_c[:], pzw[0:32, 0:W], 1.0 / H)

            def coord_half(z_in, mat_idx, tagp):
                """silu(inorm(P @ z)) -> Hw/Ww @ . ; returns [32, L] f32."""
                L = z_in.shape[1]
                ppj = psC.tile([128, 1024], f32, tag="mmC")
                nc.tensor.matmul(ppj[0:32, 0:L], cm[:, 0, :], z_in[:],
                                 start=True, stop=True)
                ppj_s = sm.tile([32, L], f32, tag="cps" + tagp)
                nc.vector.tensor_copy(ppj_s[:], ppj[0:32, 0:L])
                stp = sm.tile([32, 6], f32, tag="cst" + tagp)
                nc.vector.bn_stats(stp[:], ppj[0:32, 0:L])
                agp = sm.tile([32, 2], f32, tag="cag" + tagp)
                nc.vector.bn_aggr(agp[:], stp[:])
                vp = sm.tile([32, 1], f32, tag="cvp" + tagp)
                nc.vector.tensor_scalar_add(vp[:], agp[:, 1:2], EPS)
                rsp = sm.tile([32, 1], f32, tag="crs" + tagp)
                _newton_rsqrt(nc.vector, sm, rsp[:], vp[:], 32, "c" + tagp)
                sclp = sm.tile([32, 1], f32, tag="csc" + tagp)
                nc.vector.tensor_tensor(sclp[:], rsp[:],
                                        cp[:, CP_PG:CP_PG + 1], ALU.mult)
                nsclp = sm.tile([32, 1], f32, tag="cns" + tagp)
                nc.vector.tensor_scalar_mul(nsclp[:], sclp[:], -1.0)
                biap = sm.tile([32, 1], f32, tag="cbi" + tagp)
                nc.vector.scalar_tensor_tensor(biap[:], agp[:, 0:1], nsclp[:],
                                               cp[:, CP_PB:CP_PB + 1],
                                               ALU.mult, ALU.add)
                yt = sm.tile([32, L], f32, tag="cyt" + tagp)
                nc.vector.tensor_scalar(yt[:], ppj_s[:], sclp[:],
                                        biap[:], ALU.mult, ALU.add)
                sclh = sm.tile([32, 1], f32, tag="ch2" + tagp)
                nc.vector.tensor_scalar_mul(sclh[:], sclp[:], 0.5)
                biah = sm.tile([32, 1], f32, tag="cb2" + tagp)
                nc.vector.tensor_scalar_mul(biah[:], biap[:], 0.5)
                # silu(y) = y * (0.5 + 0.5*tanh(y/2))
                sg = sm.tile([32, L], f32, tag="csg" + tagp)
                nc.scalar.activation(sg[:], ppj_s[:], AF.Tanh,
                                     bias=biah[:], scale=sclh[:])
                nc.vector.tensor_scalar_add(sg[:], sg[:], 1.0)
                proj = sm.tile([32, L], f32r, tag="cpj" + tagp)
                nc.vector.scalar_tensor_tensor(proj[:], sg[:], 0.5, yt[:],
                                               ALU.mult, ALU.mult)
                pz = psC.tile([128, 1024], f32, tag="mmC")
                nc.tensor.matmul(pz[0:32, 0:L], cm[:, mat_idx, :], proj[:],
                                 start=True, stop=True)
                zfin = sm.tile([32, L], f32, tag="czf" + tagp)
                nc.vector.tensor_copy(zfin[:], pz[0:32, 0:L])
                return zfin

            def coord_softmax(zf, col, tagp):
                L = zf.shape[1]
                mx = sm.tile([32, 1], f32, tag="cmx" + tagp)
                nc.vector.tensor_reduce(mx[:], zf[:], axis=AX.X, op=ALU.max)
                mxn = sm.tile([32, 1], f32, tag="cmn" + tagp)
                nc.vector.tensor_scalar_mul(mxn[:], mx[:], -1.0)
                ex = sm.tile([32, L], f32, tag="cex" + tagp)
                esum = sm.tile([32, 1], f32, tag="ces" + tagp)
                nc.scalar.activation(ex[:], zf[:], AF.Exp, bias=mxn[:],
                                     scale=1.0, accum_out=esum[:])
                rec = sm.tile([32, 1], f32, tag="cre" + tagp)
                nc.vector.reciprocal(rec[:], esum[:])
                escl = sm.tile([32, 1], f32, tag="cel" + tagp)
                nc.vector.tensor_tensor(escl[:], rec[:], cp[:, col:col + 1],
                                        ALU.mult)
                a_s = sm.tile([32, L], f32, tag="cas" + tagp)
                nc.vector.tensor_scalar_mul(a_s[:], ex[:], escl[:])
                return a_s

            zh_f = coord_half(zh_c, 1, "h")
            ah = coord_softmax(zh_f, CP_AH, "h")
            zw_f = coord_half(zw_c, 2, "w")
            aw = coord_softmax(zw_f, CP_BW, "w")

            ah128 = sm.tile([128, 32], f32)
            aw128 = sm.tile([128, W], f32)
            for ch in range(NCH):
                nc.sync.dma_start(ah128[ch * 32:(ch + 1) * 32, :],
                                  ah[:, ch * 32:(ch + 1) * 32])
                nc.sync.dma_start(aw128[ch * 32:(ch + 1) * 32, :], aw[:])

            # ---- coord product (gpsimd; overlaps att path) ----
            wlin = sm.tile([128, 8, W], f32, tag="wlin")
            coord = scr.tile([128, CHS], f16, tag="coord")
            for hq in range(4):
                hs = slice(hq * 8, hq * 8 + 8)
                csl = slice(hq * 1024, (hq + 1) * 1024)
                nc.gpsimd.tensor_tensor(
                    wlin[:],
                    ah128[:, hs].unsqueeze(2).broadcast_to([128, 8, W]),
                    aw128[:].unsqueeze(1).broadcast_to([128, 8, W]),
                    ALU.add)
                nc.vector.tensor_tensor(
                    coord[:, csl], wlin[:].rearrange("p a b -> p (a b)"),
                    xcf[:, csl], ALU.mult)

            # ------------- fused -------------
            fusedraw = scr.tile([128, CHS], f16, tag="fraw")
            stF = sm.tile([128, 8, 6], f32)
            for q in range(4):
                sl = slice(q * 1024, (q + 1) * 1024)
                ps = psA.tile([128, 1024], f32, tag="mm")
                for h in range(2):
                    hsl = slice(q * 1024 + h * TS, q * 1024 + (h + 1) * TS)
                    osl = slice(h * TS, (h + 1) * TS)
                    nc.tensor.matmul(ps[:, osl], kr[:, KR_MK, :],
                                     xc[:, hsl], start=True, stop=False)
                    nc.tensor.matmul(ps[:, osl], krs[:],
                                     p1[:, hsl], start=False, stop=False)
                    nc.tensor.matmul(ps[:, osl], kr[:, KR_M15, :],
                                     p2[:, hsl], start=False, stop=False)
                    nc.tensor.matmul(ps[:, osl], kr[:, KR_FC, :],
                                     coord[:, hsl], start=False, stop=True)
                    nc.vector.bn_stats(stF[:, q * 2 + h, :], ps[:, osl])
                if q % 2 == 0:
                    nc.scalar.copy(fusedraw[:, sl], ps[:])
                else:
                    nc.vector.tensor_copy(fusedraw[:, sl], ps[:])
            c2f = _agg_c2(nc, sm, stF[:], "f")
            pjf = psC.tile([128, 1024], f32, tag="mmC")
            nc.tensor.matmul(pjf[:, 0:2], jm[:], c2f[:], start=True, stop=True)
            pjf_s = sm.tile([128, 2], f32)
            nc.vector.tensor_copy(pjf_s[:], pjf[:, 0:2])
            scl3, bia3 = _inorm_scale_bias(nc, sm, pjf_s[:, 0:1], pjf_s[:, 1:2],
                                           pp[:, PP_FUSG:PP_FUSG + 1],
                                           pp[:, PP_FUSB:PP_FUSB + 1], "3")
            # fused takes att's slot (dead after the logits matmuls)
            fused = scr.tile([128, CHS], f16, tag="att")
            fsum4 = sm.tile([128, 4], f32)
            for q in range(4):
                sl = slice(q * 1024, (q + 1) * 1024)
                nc.scalar.activation(fused[:, sl], fusedraw[:, sl], AF.Relu,
                                     bias=bia3[:], scale=scl3[:],
                                     accum_out=fsum4[:, q:q + 1])
            fsum = sm.tile([128, 1], f32)
            nc.vector.tensor_reduce(fsum[:], fsum4[:], axis=AX.X, op=ALU.add)

            # ------------- gate -> ew_g -------------
            pg = psC.tile([128, 1024], f32, tag="mmC")
            nc.tensor.matmul(pg[0:1, 0:1], fsum[:], pp[:, PP_GW:PP_GW + 1],
                             start=True, stop=True)
            u = sm.tile([1, 1], f32)
            nc.scalar.activation(u[:], pg[0:1, 0:1], AF.Exp, bias=0.0,
                                 scale=-1.0)
            nc.vector.tensor_scalar_add(u[:], u[:], 1.0)
            nc.vector.reciprocal(u[:], u[:])
            nc.vector.tensor_scalar(u[:], u[:], 1.0 - GATE_FLOOR, GATE_FLOOR,
                                    ALU.mult, ALU.add)
            pgb = psC.tile([128, 1024], f32, tag="mmC")
            nc.tensor.matmul(pgb[:, 0:1], ones_t[:], u[:], start=True,
                             stop=True)
            gate_bc = sm.tile([128, 1], f32)
            nc.vector.tensor_copy(gate_bc[:], pgb[:, 0:1])
            ew_g = wp.tile([128, 512], f16, tag="ewg")
            nc.vector.tensor_scalar_mul(ew_g[:], ew[:], gate_bc[:])

            # ------------- expand + store -------------
            # (ch, mc) blocks: 4 psC tiles -> one [128, 4096] yb -> one
            # 2MB DMA.  Drains alternate ACT/DVE; output DMAs alternate
            # ACT/DVE queues (inputs run on SP/GpSimd queues).
            evi = 0
            for ch in range(NCH):
                for mc in range(4):
                    yb = ybp.tile([128, 4, 1024], f32, tag="yb")
                    for s4 in range(4):
                        ps = psC.tile([128, 1024], f32, tag="mmC")
                        for h in range(2):
                            nc.tensor.matmul(
                                ps[:, h * TS:(h + 1) * TS],
                                ew_g[ch * 32:(ch + 1) * 32,
                                     mc * 128:(mc + 1) * 128],
                                fused[ch * 32:(ch + 1) * 32,
                                      s4 * 1024 + h * TS:
                                      s4 * 1024 + (h + 1) * TS],
                                start=True, stop=True,
                                tile_position=(ch * 32, 0))
                        # DVE copies at 2 elem/cyc vs ACT 1: give DVE 5/8
                        if evi % 8 < 3:
                            nc.scalar.copy(yb[:, s4, :], ps[:])
                        else:
                            nc.vector.tensor_copy(yb[:, s4, :], ps[:])
                        evi += 1
                    dst = y_e.ap()[mc * 128:(mc + 1) * 128,
                                   ch * 4096:(ch + 1) * 4096]
                    src = yb[:].rearrange("p a b -> p (a b)")
                    # ACT queue carries 10 output tiles, Pool the last 6
                    if ch * 4 + mc < 10:
                        nc.scalar.dma_start(dst, src)
                    else:
                        nc.gpsimd.dma_start(dst, src)

    split_multi_waits(nc)
    return nc


def _get_program():
    if "nc" not in _PROG_CACHE:
        _PROG_CACHE["nc"] = build_program()
    return _PROG_CACHE["nc"]


_PROG_CACHE = {}


def make_in_maps(inputs):
    """Host-side preprocessing: full inputs dict -> per-core in_maps."""
    gi = {k: np.asarray(v, dtype=np.float32) for k, v in inputs.items()}

    x = np.ascontiguousarray(gi["x"].reshape(NCORES, C_IN, S))

    Wc = gi["compress_w"]                       # [32, 512]
    K_ = _block_diag(gi["cot_key_w"])           # [32, 32]
    V_ = _block_diag(gi["cot_val_w"])
    A1 = np.zeros((M, 2 * M), np.float32)
    a1 = gi["cot_att1_w"]                       # [4, 8, 16]
    for g in range(G):
        A1[g * MG:(g + 1) * MG, g * 2 * MG:(g + 1) * 2 * MG] = a1[g]
    A1eff = A1[:, :M] @ K_ + A1[:, M:]
    A2 = _block_diag(gi["cot_att2_w"])
    CF = gi["cot_fuse_w"]                       # [32, 32]
    F = gi["fusion_w"]                          # [32, 64]
    Fc, Fct = F[:, :M], F[:, M:]
    M_ = Fct @ CF
    MK = M_ @ K_
    E = gi["expand_w"]                          # [512, 32]

    # compress lhsT, zero-padded per (kchunk, chunk): cw[k, kc*4+ch, ch*32+c]
    cw = np.zeros((128, 16, 128), np.float32)
    WcT = np.ascontiguousarray(Wc.T)            # [512, 32]
    for kc in range(4):
        for ch in range(NCH):
            cw[:, kc * 4 + ch, ch * 32:(ch + 1) * 32] = \
                WcT[kc * 128:(kc + 1) * 128, :]
    # expand lhsT: E.T replicated per chunk block (row tile_position)
    ewt = np.tile(np.ascontiguousarray(E.T), (NCH, 1))   # [128, 512]
    kr = np.zeros((128, 8, 128), np.float32)
    for i, mat in enumerate([A1eff, A2, V_, Fc, M_, MK,
                             (1.0 - COT_LAM) / 2.0 * M_]):
        kr[:, i, :] = _kron128(mat)
    jm = np.kron(np.ones((NCH, NCH), np.float32), np.eye(M, dtype=np.float32))
    cm = np.zeros((32, 3, 32), np.float32)
    cm[:, 0, :] = _block_diag(gi["coord_proj_w"]).T
    cm[:, 1, :] = _block_diag(gi["coord_h_w"]).T
    cm[:, 2, :] = _block_diag(gi["coord_wc_w"]).T

    pp = np.zeros((128, 8), np.float32)
    rep = lambda v: np.tile(np.asarray(v, np.float32), NCH)
    pp[:, PP_INCG] = rep(gi["inc_g"])
    pp[:, PP_INCB] = rep(gi["inc_b"])
    pp[:, PP_ATTG] = rep(gi["cot_attn_g"])
    pp[:, PP_ATTB] = rep(gi["cot_attn_b"])
    pp[:, PP_FUSG] = rep(gi["fus_g"])
    pp[:, PP_FUSB] = rep(gi["fus_b"])
    pp[:, PP_GW] = rep(gi["gate_w"].reshape(-1) / float(S))

    cp = np.zeros((32, 6), np.float32)
    cp[:, CP_PG] = gi["coord_proj_g"]
    cp[:, CP_PB] = gi["coord_proj_b"]
    cp[:, CP_AH] = float(gi["coord_alpha"]) * H
    cp[:, CP_BW] = float(gi["coord_beta"]) * W

    on = np.ones((1, 128), np.float32)

    shared = dict(cw=cw, ew=ewt.astype(np.float16), kr=kr.astype(np.float16),
                  krm=_kron128(M_).astype(np_bf16),
                  jm=jm, cm=cm, pp=pp, cp=cp, on=on)
    return [dict(shared, x=np.ascontiguousarray(x[i]))
            for i in range(NCORES)]


def kernel(**inputs):
    nc = _get_program()
    in_maps = make_in_maps(inputs)
    res = run_bass_kernel_spmd(nc, in_maps, list(range(NCORES)))
    out = np.stack([res.results[i]["y"].reshape(C_IN, H, W)
                    for i in range(NCORES)], axis=0)
    return out.astype(np.float32)
